# revision 6
# baseline (speedup 1.0000x reference)
"""Trainium2 Bass kernel for nn_ConstrainedLayer (elementwise QP clip).

reference:  out = clip(pred, min(-9*y, 11*y), max(-9*y, 11*y))

Pure data-parallel over batch: 16777216 elements split across 8 NeuronCores
(2097152 each), each core streams its chunk through SBUF in [128 x F] tiles.

Per tile (bit-exact vs the jax reference):
  ACT : b  = 11 * y                     (activation Copy, scale=11)
  DVE : lo = (y * -9) min b             (fused scalar_tensor_tensor)
  DVE : hi = (y * -9) max b             (fused scalar_tensor_tensor)
  DVE : t  = max(p, lo)
  DVE : o  = min(t, hi)

Memory-bound problem: 3 x 8 MiB HBM traffic per core ~= 70 us at ~358 GB/s.
"""

import sys

import numpy as np

for _p in ("/opt/trn_rl_repo", "/root/.axon_site/_ro/trn_rl_repo"):
    if _p not in sys.path:
        sys.path.append(_p)

N = 16777216
N_CORES = 8
PER_CORE = N // N_CORES  # 2097152
P = 128
F = 2048
T = PER_CORE // (P * F)  # 8 tiles per core

_CACHE = {}


def _build_nc():
    import concourse.bacc as bacc
    import concourse.tile as tile
    from concourse import mybir

    f32 = mybir.dt.float32
    Alu = mybir.AluOpType

    # Bacc (not raw Bass): its compile pass splits multi-sem sync waits into
    # event semaphores — walrus codegen allows only 1 wait per instruction.
    nc = bacc.Bacc(
        "TRN2", target_bir_lowering=False, debug=False, num_devices=N_CORES
    )
    pred = nc.declare_dram_parameter("predictions", [T, P, F], f32, isOutput=False)
    y = nc.declare_dram_parameter("y_true_batch", [T, P, F], f32, isOutput=False)
    out = nc.declare_dram_parameter("out", [T, P, F], f32, isOutput=True)

    with tile.TileContext(nc) as tc:
        with (
            tc.tile_pool(name="io", bufs=3) as io_pool,
            tc.tile_pool(name="tmp", bufs=2) as tmp_pool,
        ):
            for i in range(T):
                tp = io_pool.tile([P, F], f32, tag="tp")
                nc.sync.dma_start(tp[:], pred[i])
                ty = io_pool.tile([P, F], f32, tag="ty")
                nc.sync.dma_start(ty[:], y[i])

                a = tmp_pool.tile([P, F], f32, tag="a")
                nc.scalar.activation(
                    a[:], ty[:], mybir.ActivationFunctionType.Copy, scale=-9.0
                )
                b = tmp_pool.tile([P, F], f32, tag="b")
                nc.scalar.activation(
                    b[:], ty[:], mybir.ActivationFunctionType.Copy, scale=11.0
                )
                lo = tmp_pool.tile([P, F], f32, tag="lo")
                nc.vector.tensor_tensor(lo[:], a[:], b[:], op=Alu.min)
                hi = tmp_pool.tile([P, F], f32, tag="hi")
                nc.vector.tensor_tensor(hi[:], a[:], b[:], op=Alu.max)
                t = tmp_pool.tile([P, F], f32, tag="t")
                nc.vector.tensor_tensor(t[:], tp[:], lo[:], op=Alu.max)
                o = tmp_pool.tile([P, F], f32, tag="o")
                nc.vector.tensor_tensor(o[:], t[:], hi[:], op=Alu.min)

                # store on the ACT HWDGE ring so stores don't head-of-line
                # block the loads issued on the sync ring
                nc.scalar.dma_start(out[i], o[:])
    nc.finalize()
    return nc


def _get_nc():
    if "nc" not in _CACHE:
        _CACHE["nc"] = _build_nc()
    return _CACHE["nc"]


def _get_executor():
    """Cached jitted SPMD executor over 8 cores (mirrors
    bass2jax.run_bass_via_pjrt multi-core branch, built once so repeat calls
    don't re-trace)."""
    if "exec" in _CACHE:
        return _CACHE["exec"]

    import jax
    from jax.experimental.shard_map import shard_map
    from jax.sharding import Mesh, NamedSharding, PartitionSpec

    from concourse import mybir
    from concourse.bass2jax import (
        _bass_exec_p,
        install_neuronx_cc_hook,
        partition_id_tensor,
    )

    nc = _get_nc()
    install_neuronx_cc_hook()

    partition_name = nc.partition_id_tensor.name if nc.partition_id_tensor else None

    in_names = []
    out_names = []
    out_avals = []
    zero_outs = []
    for alloc in nc.m.functions[0].allocations:
        if not isinstance(alloc, mybir.MemoryLocationSet):
            continue
        name = alloc.memorylocations[0].name
        if alloc.kind == "ExternalInput":
            if name != partition_name:
                in_names.append(name)
        elif alloc.kind == "ExternalOutput":
            out_names.append(name)
            shape = tuple(alloc.tensor_shape)
            dtype = mybir.dt.np(alloc.dtype)
            out_avals.append(jax.core.ShapedArray(shape, dtype))
            zero_outs.append(np.zeros(shape, dtype))
    n_params = len(in_names)
    all_in_names = tuple(in_names) + tuple(out_names)
    if partition_name is not None:
        all_in_names = all_in_names + (partition_name,)

    def _body(*args):
        operands = list(args)
        if partition_name is not None:
            operands.append(partition_id_tensor())
        outs = _bass_exec_p.bind(
            *operands,
            out_avals=tuple(out_avals),
            in_names=all_in_names,
            out_names=tuple(out_names),
            lowering_input_output_aliases=(),
            sim_require_finite=True,
            sim_require_nnan=True,
            nc=nc,
        )
        return tuple(outs)

    devices = jax.devices()[:N_CORES]
    mesh = Mesh(np.asarray(devices), ("core",))
    spec = PartitionSpec("core")
    n_args = n_params + len(out_names)
    sharded = jax.jit(
        shard_map(
            _body,
            mesh=mesh,
            in_specs=(spec,) * n_args,
            out_specs=(spec,) * len(out_names),
            check_rep=False,
        ),
        keep_unused=True,
    )
    sharding = NamedSharding(mesh, spec)
    zeros_dev = [
        jax.device_put(np.zeros((N_CORES * z.shape[0], *z.shape[1:]), z.dtype), sharding)
        for z in zero_outs
    ]
    _CACHE["exec"] = (sharded, sharding, in_names, zeros_dev)
    return _CACHE["exec"]


def _to_core_shape(arr):
    return np.ascontiguousarray(np.asarray(arr, dtype=np.float32)).reshape(
        N_CORES * T, P, F
    )


def kernel(predictions, y_true_batch):
    import jax

    sharded, sharding, in_names, zeros_dev = _get_executor()
    by_name = {"predictions": predictions, "y_true_batch": y_true_batch}
    args = [
        jax.device_put(_to_core_shape(by_name[n]), sharding) for n in in_names
    ] + zeros_dev
    (out,) = sharded(*args)
    return np.asarray(out).reshape(N, 1)


def benchmark(predictions, y_true_batch, iters=10):
    """Times repeat executions with device-resident inputs.
    Returns (output, list of per-iteration wall seconds)."""
    import time

    import jax

    sharded, sharding, in_names, zeros_dev = _get_executor()
    by_name = {"predictions": predictions, "y_true_batch": y_true_batch}
    args = [
        jax.device_put(_to_core_shape(by_name[n]), sharding) for n in in_names
    ] + zeros_dev
    (out,) = sharded(*args)  # warmup + compile
    out.block_until_ready()
    times = []
    for _ in range(iters):
        t0 = time.perf_counter()
        (o,) = sharded(*args)
        o.block_until_ready()
        times.append(time.perf_counter() - t0)
    return np.asarray(out).reshape(N, 1), times


def predict_timeline():
    """Offline cost-model makespan estimate (ns) for one core."""
    from concourse.timeline_sim import TimelineSim

    return TimelineSim(_get_nc()).simulate()


# revision 13
# speedup vs baseline: 189.6199x; 189.6199x over previous
"""Trainium2 Bass kernel for nn_ConstrainedLayer (elementwise QP clip).

reference:  out = clip(pred, min(-9*y, 11*y), max(-9*y, 11*y))

Pure data-parallel over batch: 16777216 elements split across 8 NeuronCores
(2097152 each); each core streams its chunk through SBUF as 8 tiles of
[128 x 2048] f32, triple-buffered, loads on the sync HWDGE ring and stores on
the scalar HWDGE ring so the streams don't share one FIFO.

Per tile (bit-exact vs the jax reference -- every op is single-rounding IEEE):
  ACT : a  = -9 * y    (activation Copy, scale=-9)
  ACT : b  = 11 * y    (activation Copy, scale=11)
  DVE : lo = min(a, b)
  DVE : hi = max(a, b)
  DVE : t  = max(p, lo)
  DVE : o  = min(t, hi)

Memory-bound problem: 3 x 8 MiB HBM traffic per core ~= 70 us at ~358 GB/s;
DVE does 4 full-tensor tensor_tensor passes ~= 73 us, so the two engines are
co-bottlenecked.  Measured per-pass device time ~= 84 us/core (reps-slope
method), vs 86 us predicted by the concourse TimelineSim cost model.
"""

import sys

import numpy as np

for _p in ("/opt/trn_rl_repo", "/root/.axon_site/_ro/trn_rl_repo"):
    if _p not in sys.path:
        sys.path.append(_p)

N = 16777216
N_CORES = 8
PER_CORE = N // N_CORES  # 2097152
P = 128
F = 2048
T = PER_CORE // (P * F)  # 8 tiles per core

_CACHE = {}


def _build_nc():
    import concourse.bacc as bacc
    import concourse.tile as tile
    from concourse import mybir

    f32 = mybir.dt.float32
    Alu = mybir.AluOpType

    # Bacc (not raw Bass): its compile pass splits multi-sem sync waits into
    # event semaphores — walrus codegen allows only 1 wait per instruction.
    nc = bacc.Bacc(
        "TRN2", target_bir_lowering=False, debug=False, num_devices=N_CORES
    )
    pred = nc.declare_dram_parameter("predictions", [T, P, F], f32, isOutput=False)
    y = nc.declare_dram_parameter("y_true_batch", [T, P, F], f32, isOutput=False)
    out = nc.declare_dram_parameter("out", [T, P, F], f32, isOutput=True)

    with tile.TileContext(nc) as tc:
        with (
            tc.tile_pool(name="io", bufs=3) as io_pool,
            tc.tile_pool(name="tmp", bufs=2) as tmp_pool,
        ):
            for i in range(T):
                tp = io_pool.tile([P, F], f32, tag="tp")
                nc.sync.dma_start(tp[:], pred[i])
                ty = io_pool.tile([P, F], f32, tag="ty")
                nc.sync.dma_start(ty[:], y[i])

                a = tmp_pool.tile([P, F], f32, tag="a")
                nc.scalar.activation(
                    a[:], ty[:], mybir.ActivationFunctionType.Copy, scale=-9.0
                )
                b = tmp_pool.tile([P, F], f32, tag="b")
                nc.scalar.activation(
                    b[:], ty[:], mybir.ActivationFunctionType.Copy, scale=11.0
                )
                lo = tmp_pool.tile([P, F], f32, tag="lo")
                nc.vector.tensor_tensor(lo[:], a[:], b[:], op=Alu.min)
                hi = tmp_pool.tile([P, F], f32, tag="hi")
                nc.vector.tensor_tensor(hi[:], a[:], b[:], op=Alu.max)
                t = tmp_pool.tile([P, F], f32, tag="t")
                nc.vector.tensor_tensor(t[:], tp[:], lo[:], op=Alu.max)
                o = tmp_pool.tile([P, F], f32, tag="o")
                nc.vector.tensor_tensor(o[:], t[:], hi[:], op=Alu.min)

                # store on the ACT HWDGE ring so stores don't head-of-line
                # block the loads issued on the sync ring
                nc.scalar.dma_start(out[i], o[:])
    nc.finalize()
    return nc


def _get_nc():
    if "nc" not in _CACHE:
        _CACHE["nc"] = _build_nc()
    return _CACHE["nc"]


def _get_executor():
    """Cached jitted SPMD executor over 8 cores (mirrors
    bass2jax.run_bass_via_pjrt multi-core branch, built once so repeat calls
    don't re-trace)."""
    if "exec" in _CACHE:
        return _CACHE["exec"]

    import jax
    from jax.sharding import Mesh, NamedSharding, PartitionSpec

    def shard_map(f, **kw):
        try:
            from jax.experimental.shard_map import shard_map as sm

            return sm(f, **kw)
        except (ImportError, TypeError):
            kw["check_vma"] = kw.pop("check_rep", False)
            return jax.shard_map(f, **kw)

    from concourse import mybir
    from concourse.bass2jax import (
        _bass_exec_p,
        install_neuronx_cc_hook,
        partition_id_tensor,
    )

    nc = _get_nc()
    install_neuronx_cc_hook()

    partition_name = nc.partition_id_tensor.name if nc.partition_id_tensor else None

    in_names = []
    out_names = []
    out_avals = []
    zero_outs = []
    for alloc in nc.m.functions[0].allocations:
        if not isinstance(alloc, mybir.MemoryLocationSet):
            continue
        name = alloc.memorylocations[0].name
        if alloc.kind == "ExternalInput":
            if name != partition_name:
                in_names.append(name)
        elif alloc.kind == "ExternalOutput":
            out_names.append(name)
            shape = tuple(alloc.tensor_shape)
            dtype = mybir.dt.np(alloc.dtype)
            out_avals.append(jax.core.ShapedArray(shape, dtype))
            zero_outs.append(np.zeros(shape, dtype))
    n_params = len(in_names)
    all_in_names = tuple(in_names) + tuple(out_names)
    if partition_name is not None:
        all_in_names = all_in_names + (partition_name,)

    def _body(*args):
        operands = list(args)
        if partition_name is not None:
            operands.append(partition_id_tensor())
        outs = _bass_exec_p.bind(
            *operands,
            out_avals=tuple(out_avals),
            in_names=all_in_names,
            out_names=tuple(out_names),
            lowering_input_output_aliases=(),
            sim_require_finite=True,
            sim_require_nnan=True,
            nc=nc,
        )
        return tuple(outs)

    devices = jax.devices()[:N_CORES]
    mesh = Mesh(np.asarray(devices), ("core",))
    spec = PartitionSpec("core")
    n_args = n_params + len(out_names)
    sharded = jax.jit(
        shard_map(
            _body,
            mesh=mesh,
            in_specs=(spec,) * n_args,
            out_specs=(spec,) * len(out_names),
            check_rep=False,
        ),
        keep_unused=True,
    )
    sharding = NamedSharding(mesh, spec)
    zeros_dev = [
        jax.device_put(np.zeros((N_CORES * z.shape[0], *z.shape[1:]), z.dtype), sharding)
        for z in zero_outs
    ]
    _CACHE["exec"] = (sharded, sharding, in_names, zeros_dev)
    return _CACHE["exec"]


def _to_core_shape(arr):
    return np.ascontiguousarray(np.asarray(arr, dtype=np.float32)).reshape(
        N_CORES * T, P, F
    )


def kernel(predictions, y_true_batch):
    import jax

    sharded, sharding, in_names, zeros_dev = _get_executor()
    by_name = {"predictions": predictions, "y_true_batch": y_true_batch}
    args = [
        jax.device_put(_to_core_shape(by_name[n]), sharding) for n in in_names
    ] + zeros_dev
    (out,) = sharded(*args)
    return np.asarray(out).reshape(N, 1)


def benchmark(predictions, y_true_batch, iters=10):
    """Times repeat executions with device-resident inputs.
    Returns (output, list of per-iteration wall seconds)."""
    import time

    import jax

    sharded, sharding, in_names, zeros_dev = _get_executor()
    by_name = {"predictions": predictions, "y_true_batch": y_true_batch}
    args = [
        jax.device_put(_to_core_shape(by_name[n]), sharding) for n in in_names
    ] + zeros_dev
    (out,) = sharded(*args)  # warmup + compile
    out.block_until_ready()
    times = []
    for _ in range(iters):
        t0 = time.perf_counter()
        (o,) = sharded(*args)
        o.block_until_ready()
        times.append(time.perf_counter() - t0)
    return np.asarray(out).reshape(N, 1), times


def predict_timeline():
    """Offline cost-model makespan estimate (ns) for one core."""
    from concourse.timeline_sim import TimelineSim

    return TimelineSim(_get_nc()).simulate()


# revision 16
# speedup vs baseline: 1274.0466x; 6.7189x over previous
"""Trainium2 Bass kernel for nn_ConstrainedLayer (elementwise QP clip).

reference:  out = clip(pred, min(-9*y, 11*y), max(-9*y, 11*y))

Pure data-parallel over batch: 16777216 elements split across 8 NeuronCores
(2097152 each); each core streams its chunk through SBUF as 8 tiles of
[128 x 2048] f32, triple-buffered.  DMA is balanced across the two HWDGE
FIFOs (p-loads on the sync ring, y-loads on the scalar ring, stores
alternating) so each ring carries ~12.6 MB per pass -- HW-measured ~7 us/pass
faster than putting all 16 loads on one ring.

Per tile (bit-exact vs the jax reference -- every op is single-rounding IEEE):
  ACT : a  = -9 * y    (activation Copy, scale=-9)
  ACT : b  = 11 * y    (activation Copy, scale=11)
  DVE : lo = min(a, b)
  DVE : hi = max(a, b)
  DVE : t  = max(p, lo)
  DVE : o  = min(t, hi)

Memory-bound problem: 3 x 8 MiB HBM traffic per core ~= 70 us at ~358 GB/s;
DVE does 4 full-tensor tensor_tensor passes ~= 73 us, so the two engines are
co-bottlenecked.  Measured per-pass device time ~= 84 us/core (reps-slope
method), vs 86 us predicted by the concourse TimelineSim cost model.
"""

import sys

import numpy as np

for _p in ("/opt/trn_rl_repo", "/root/.axon_site/_ro/trn_rl_repo"):
    if _p not in sys.path:
        sys.path.append(_p)

N = 16777216
N_CORES = 8
PER_CORE = N // N_CORES  # 2097152
P = 128
F = 2048
T = PER_CORE // (P * F)  # 8 tiles per core

_CACHE = {}


def _build_nc():
    import concourse.bacc as bacc
    import concourse.tile as tile
    from concourse import mybir

    f32 = mybir.dt.float32
    Alu = mybir.AluOpType

    # Bacc (not raw Bass): its compile pass splits multi-sem sync waits into
    # event semaphores — walrus codegen allows only 1 wait per instruction.
    nc = bacc.Bacc(
        "TRN2", target_bir_lowering=False, debug=False, num_devices=N_CORES
    )
    pred = nc.declare_dram_parameter("predictions", [T, P, F], f32, isOutput=False)
    y = nc.declare_dram_parameter("y_true_batch", [T, P, F], f32, isOutput=False)
    out = nc.declare_dram_parameter("out", [T, P, F], f32, isOutput=True)

    with tile.TileContext(nc) as tc:
        with (
            tc.tile_pool(name="io", bufs=3) as io_pool,
            tc.tile_pool(name="tmp", bufs=2) as tmp_pool,
        ):
            for i in range(T):
                # balance the two HWDGE FIFOs: p-loads on the sync ring,
                # y-loads on the scalar ring, stores alternating -- ~12.6 MB
                # per ring per pass instead of 16.8/8.4 (HW-measured ~7 us/pass
                # faster than all-loads-on-sync)
                tp = io_pool.tile([P, F], f32, tag="tp")
                nc.sync.dma_start(tp[:], pred[i])
                ty = io_pool.tile([P, F], f32, tag="ty")
                nc.scalar.dma_start(ty[:], y[i])

                a = tmp_pool.tile([P, F], f32, tag="a")
                nc.scalar.activation(
                    a[:], ty[:], mybir.ActivationFunctionType.Copy, scale=-9.0
                )
                b = tmp_pool.tile([P, F], f32, tag="b")
                nc.scalar.activation(
                    b[:], ty[:], mybir.ActivationFunctionType.Copy, scale=11.0
                )
                lo = tmp_pool.tile([P, F], f32, tag="lo")
                nc.vector.tensor_tensor(lo[:], a[:], b[:], op=Alu.min)
                hi = tmp_pool.tile([P, F], f32, tag="hi")
                nc.vector.tensor_tensor(hi[:], a[:], b[:], op=Alu.max)
                t = tmp_pool.tile([P, F], f32, tag="t")
                nc.vector.tensor_tensor(t[:], tp[:], lo[:], op=Alu.max)
                o = tmp_pool.tile([P, F], f32, tag="o")
                nc.vector.tensor_tensor(o[:], t[:], hi[:], op=Alu.min)

                st = nc.sync if i % 2 == 0 else nc.scalar
                st.dma_start(out[i], o[:])
    nc.finalize()
    return nc


def _get_nc():
    if "nc" not in _CACHE:
        _CACHE["nc"] = _build_nc()
    return _CACHE["nc"]


def _get_executor():
    """Cached jitted SPMD executor over 8 cores (mirrors
    bass2jax.run_bass_via_pjrt multi-core branch, built once so repeat calls
    don't re-trace)."""
    if "exec" in _CACHE:
        return _CACHE["exec"]

    import jax
    from jax.sharding import Mesh, NamedSharding, PartitionSpec

    def shard_map(f, **kw):
        try:
            from jax.experimental.shard_map import shard_map as sm

            return sm(f, **kw)
        except (ImportError, TypeError):
            kw["check_vma"] = kw.pop("check_rep", False)
            return jax.shard_map(f, **kw)

    from concourse import mybir
    from concourse.bass2jax import (
        _bass_exec_p,
        install_neuronx_cc_hook,
        partition_id_tensor,
    )

    nc = _get_nc()
    install_neuronx_cc_hook()

    partition_name = nc.partition_id_tensor.name if nc.partition_id_tensor else None

    in_names = []
    out_names = []
    out_avals = []
    zero_outs = []
    for alloc in nc.m.functions[0].allocations:
        if not isinstance(alloc, mybir.MemoryLocationSet):
            continue
        name = alloc.memorylocations[0].name
        if alloc.kind == "ExternalInput":
            if name != partition_name:
                in_names.append(name)
        elif alloc.kind == "ExternalOutput":
            out_names.append(name)
            shape = tuple(alloc.tensor_shape)
            dtype = mybir.dt.np(alloc.dtype)
            out_avals.append(jax.core.ShapedArray(shape, dtype))
            zero_outs.append(np.zeros(shape, dtype))
    n_params = len(in_names)
    all_in_names = tuple(in_names) + tuple(out_names)
    if partition_name is not None:
        all_in_names = all_in_names + (partition_name,)

    def _body(*args):
        operands = list(args)
        if partition_name is not None:
            operands.append(partition_id_tensor())
        outs = _bass_exec_p.bind(
            *operands,
            out_avals=tuple(out_avals),
            in_names=all_in_names,
            out_names=tuple(out_names),
            lowering_input_output_aliases=(),
            sim_require_finite=True,
            sim_require_nnan=True,
            nc=nc,
        )
        return tuple(outs)

    devices = jax.devices()[:N_CORES]
    mesh = Mesh(np.asarray(devices), ("core",))
    spec = PartitionSpec("core")
    n_args = n_params + len(out_names)
    sharded = jax.jit(
        shard_map(
            _body,
            mesh=mesh,
            in_specs=(spec,) * n_args,
            out_specs=(spec,) * len(out_names),
            check_rep=False,
        ),
        keep_unused=True,
    )
    sharding = NamedSharding(mesh, spec)
    zeros_dev = [
        jax.device_put(np.zeros((N_CORES * z.shape[0], *z.shape[1:]), z.dtype), sharding)
        for z in zero_outs
    ]
    _CACHE["exec"] = (sharded, sharding, in_names, zeros_dev)
    return _CACHE["exec"]


def _to_core_shape(arr):
    return np.ascontiguousarray(np.asarray(arr, dtype=np.float32)).reshape(
        N_CORES * T, P, F
    )


def kernel(predictions, y_true_batch):
    import jax

    sharded, sharding, in_names, zeros_dev = _get_executor()
    by_name = {"predictions": predictions, "y_true_batch": y_true_batch}
    args = [
        jax.device_put(_to_core_shape(by_name[n]), sharding) for n in in_names
    ] + zeros_dev
    (out,) = sharded(*args)
    return np.asarray(out).reshape(N, 1)


def benchmark(predictions, y_true_batch, iters=10):
    """Times repeat executions with device-resident inputs.
    Returns (output, list of per-iteration wall seconds)."""
    import time

    import jax

    sharded, sharding, in_names, zeros_dev = _get_executor()
    by_name = {"predictions": predictions, "y_true_batch": y_true_batch}
    args = [
        jax.device_put(_to_core_shape(by_name[n]), sharding) for n in in_names
    ] + zeros_dev
    (out,) = sharded(*args)  # warmup + compile
    out.block_until_ready()
    times = []
    for _ in range(iters):
        t0 = time.perf_counter()
        (o,) = sharded(*args)
        o.block_until_ready()
        times.append(time.perf_counter() - t0)
    return np.asarray(out).reshape(N, 1), times


def predict_timeline():
    """Offline cost-model makespan estimate (ns) for one core."""
    from concourse.timeline_sim import TimelineSim

    return TimelineSim(_get_nc()).simulate()
